# revision 11
# baseline (speedup 1.0000x reference)
"""Trainium2 Bass kernel for nn_AttnBlock (GroupNorm + single-head spatial
self-attention + residual), SPMD over 8 NeuronCores.

Sharding: data-parallel over batch B=4, x2 split over query tokens
(each core handles 2048 of the 4096 spatial tokens of one batch).
The per-core input x-slab is permuted so that the core's own query tokens
come first -> the SPMD program is identical on every core (softmax/GN are
permutation-invariant over tokens).

Device algebra (weights fused on host, fp64):
  xn = GroupNorm(x) * gn_w + gn_b                      [C, N] per batch
  scoresT[j,i] = sum_c' xn[c',j] * r[c',i],  r = W1 xn + rb
      where W1 = Wk^T Wq, rb = Wk^T bq  (bk shifts all logits of a query
      equally and cancels in softmax -> dropped exactly)
  e = exp(scoresT * C^-0.5)      (no max-subtraction needed: logits ~ N(0,1))
  u[c',i] = sum_j xnT[j,c'] e[j,i];  denom[i] = sum_j e[j,i]  (ones-matmul)
  out = W2 (u * 1/denom) + b2 + x,  W2 = Wp Wv, b2 = Wp bv + bp
"""

import os
import sys

for _p in ("/opt/trn_rl_repo", "/root/.axon_site/_ro/trn_rl_repo"):
    if os.path.isdir(_p) and _p not in sys.path:
        sys.path.insert(0, _p)

import numpy as np

B, C, H, W = 4, 512, 64, 64
N = H * W            # 4096 tokens
NQ = N // 2          # 2048 query tokens per core
T = C // 128         # 4 channel tiles
JT = N // 128        # 32 key tiles
IG = NQ // 512       # 4 query groups of 512
NUM_GROUPS = 32
EPS = 1e-5
SCALE = float(C) ** -0.5

# matmul compute dtype for the heavy matmuls:
#   "fp8"  - fastest: fp8e4m3 with DoubleRow (2 k-subtiles/instr, 0.5 cyc/row);
#            rel err ~8e-3 (default)
#   "bf16" - rel err ~5e-4
#   "f32r" - ~13% slower than bf16 (self-loading 4-byte weights); ~2e-5
#   "f32"  - exact but 4 cycles/row on the PE
MM_DTYPE = os.environ.get("BASS_MM_DTYPE", "fp8")
MSHIFT = 3.0  # constant logit shift before exp (softmax-invariant); keeps
              # e = exp(s*SCALE - MSHIFT) <= ~e^2.8 within fp8e4m3 range (240)

_PROGRAM_CACHE = {}
LAST_RESULTS = None

# walrus disables its LDWEIGHTS optimization (incl. fast-weight-load) by
# default; re-enable it for this kernel unless BASS_LDW_OPT=0.
_LDW_PATCHED = False


def _patch_ldw_opt():
    global _LDW_PATCHED
    if _LDW_PATCHED or os.environ.get("BASS_LDW_OPT", "0") == "0":
        return
    from concourse import bass_utils as _bu

    _orig = _bu.run_command

    def _patched(argv, **kw):
        argv = [
            ("--enable-ldw-opt=true" if a == "--enable-ldw-opt=false" else a)
            for a in argv
        ]
        return _orig(argv, **kw)

    _bu.run_command = _patched
    _LDW_PATCHED = True


def _build_program(mm_dtype_name: str, repeat: int = 1, denom_mm: bool = False,
                   dma_transpose: bool = False, psa: int = 2, psp: int = 2,
                   ebufs: int = 3, dual_dma: bool = True):
    _patch_ldw_opt()
    import concourse.bass as bass
    import concourse.tile as tile
    from concourse import bacc, mybir

    f32 = mybir.dt.float32
    mm_dt = {"f32": None, "f32r": mybir.dt.float32r, "bf16": mybir.dt.bfloat16}[mm_dtype_name]
    AF = mybir.ActivationFunctionType
    OP = mybir.AluOpType

    nc = bacc.Bacc("TRN2")
    mdt_early = f32 if mm_dt is None else mm_dt

    xb_d = nc.declare_dram_parameter("xb", [C, N], f32, isOutput=False)
    # x stays fp32 on the GN path: loading it bf16 saves ~6us of prologue but
    # doubles the end-to-end error (5.5e-4 -> 1.1e-3); not worth the margin.
    xdt = f32
    w1t_d = nc.declare_dram_parameter("w1t", [C, C], mdt_early, isOutput=False)
    w2t_d = nc.declare_dram_parameter("w2t", [C, C], mdt_early, isOutput=False)
    cv_d = nc.declare_dram_parameter("cvec", [128, 4, T], f32, isOutput=False)
    gi_d = nc.declare_dram_parameter("gi", [128, 8], f32, isOutput=False)
    git_d = nc.declare_dram_parameter("git", [8, 128], f32, isOutput=False)
    id_d = nc.declare_dram_parameter("ident", [128, 128], mdt_early, isOutput=False)
    on_d = nc.declare_dram_parameter("onesc", [128, 1], f32, isOutput=False)
    onr_d = nc.declare_dram_parameter("onesr", [1, 128], f32, isOutput=False)
    out_d = nc.declare_dram_parameter("out", [C, NQ], f32, isOutput=True)

    mdt = mdt_early

    def mm(out, lhsT, rhs, start, stop):
        nc.tensor.matmul(out, lhsT, rhs, start=start, stop=stop)

    with tile.TileContext(nc) as tc:
        with (
            tc.tile_pool(name="big", bufs=2) as pbig,
            tc.tile_pool(name="const", bufs=1) as pc,
            tc.tile_pool(name="stat", bufs=2) as pst,
            tc.tile_pool(name="rpool", bufs=1) as prr,
            tc.tile_pool(name="upool", bufs=1) as puu,
            tc.tile_pool(name="epool", bufs=ebufs) as pee,
            tc.tile_pool(name="iopool", bufs=2) as pio,
            tc.tile_pool(name="psA", bufs=psa, space="PSUM") as ppA,
            tc.tile_pool(name="psU", bufs=4, space="PSUM") as ppU,
            tc.tile_pool(name="psP", bufs=psp, space="PSUM") as ppP,
        ):
            # ---- constant / weight loads ----
            weng = nc.scalar if dual_dma else nc.sync
            W1T = pc.tile([128, T, C], mdt)
            weng.dma_start(out=W1T, in_=w1t_d[:].rearrange("(t p) f -> p t f", p=128))
            W2T = pc.tile([128, T, C], mdt)
            weng.dma_start(out=W2T, in_=w2t_d[:].rearrange("(t p) f -> p t f", p=128))
            GI = pc.tile([128, 8], f32)
            nc.sync.dma_start(out=GI, in_=gi_d[:])
            GIT = pc.tile([8, 128], f32)
            nc.sync.dma_start(out=GIT, in_=git_d[:])
            IDENT = pc.tile([128, 128], mdt)
            nc.sync.dma_start(out=IDENT, in_=id_d[:])
            CV = pc.tile([128, 4, T], f32)
            nc.sync.dma_start(out=CV, in_=cv_d[:])
            RB, B2, GNW, GNB = CV[:, 0, :], CV[:, 1, :], CV[:, 2, :], CV[:, 3, :]
            ONES = pc.tile([128, 1], f32)
            nc.sync.dma_start(out=ONES, in_=on_d[:])
            ONESM = pc.tile([128, 1], mdt)
            nc.vector.tensor_copy(ONESM, ONES)
            ONESR = pc.tile([1, 128], f32)
            nc.sync.dma_start(out=ONESR, in_=onr_d[:])
            SS = pc.tile([128, T, 2], f32)  # per-channel (scale, shift)

            # ---- load x + per-tile GroupNorm (stats pipeline behind DMA) ----
            # each 16-channel group lives inside one 128-channel tile, so
            # stats -> normalize proceed per tile without a global join
            X = pbig.tile([128, T, N], xdt, tag="big")
            XN = pbig.tile([128, T, N], mdt, tag="big")
            # group-0 r accumulates per tile inside the GN loop, borrowing the
            # psU banks (idle until the first attn-V matmul)
            Pr0 = [ppU.tile([128, 512], f32, tag="Pu", name=f"Pr0_{m}") for m in range(T)]
            xb_t = xb_d[:].rearrange("(t p) n -> p t n", p=128)
            epsT = pc.tile([8, 1], f32)
            nc.vector.memset(epsT, EPS)
            for t in range(T):
                for h in range(4):
                    eng = nc.scalar if (dual_dma and h % 2 == 1) else nc.sync
                    eng.dma_start(
                        out=X[:, t, 1024 * h : 1024 * (h + 1)],
                        in_=xb_t[:, t, 1024 * h : 1024 * (h + 1)],
                    )
                stats_t = pst.tile([128, 8, 6], f32, tag="stats")
                for s in range(8):
                    nc.vector.bn_stats(out=stats_t[:, s, :], in_=X[:, t, 512 * s : 512 * (s + 1)])
                mv_t = pst.tile([128, 2], f32, tag="mv")
                nc.vector.bn_aggr(out=mv_t, in_=stats_t)
                perch_t = pst.tile([128, 2], f32, tag="perch")  # (mean, E[x^2])
                nc.vector.tensor_copy(perch_t[:, 0:1], mv_t[:, 0:1])
                nc.vector.tensor_mul(perch_t[:, 1:2], mv_t[:, 0:1], mv_t[:, 0:1])
                nc.vector.tensor_add(perch_t[:, 1:2], perch_t[:, 1:2], mv_t[:, 1:2])
                GSp = ppA.tile([8, 2], f32, tag="psA", name=f"GSp{t}")
                nc.tensor.matmul(GSp, GI, perch_t, start=True, stop=True)
                GB = pst.tile([8, 2], f32, tag="GB")  # (mean_g, rstd_g)
                tmpg = pst.tile([8, 1], f32, tag="tmpg")
                nc.vector.tensor_copy(GB, GSp)
                nc.vector.tensor_mul(tmpg, GB[:, 0:1], GB[:, 0:1])
                nc.vector.tensor_sub(GB[:, 1:2], GB[:, 1:2], tmpg)  # var_g
                nc.scalar.activation(GB[:, 1:2], GB[:, 1:2], AF.Sqrt, bias=epsT)
                nc.vector.reciprocal(GB[:, 1:2], GB[:, 1:2])
                PB = ppA.tile([128, 2], f32, tag="psA", name=f"PB{t}")
                nc.tensor.matmul(PB, GIT, GB, start=True, stop=True)
                tmpc = pst.tile([128, 1], f32, tag="tmpc")
                nc.vector.tensor_mul(SS[:, t, 0:1], PB[:, 1:2], GNW[:, t : t + 1])
                nc.vector.tensor_mul(tmpc, PB[:, 0:1], SS[:, t, 0:1])
                nc.vector.tensor_sub(SS[:, t, 1:2], GNB[:, t : t + 1], tmpc)
                # xn = x*scale + shift (two halves: first unblocks attention)
                for h in range(2):
                    nc.vector.tensor_scalar(
                        out=XN[:, t, 2048 * h : 2048 * (h + 1)],
                        in0=X[:, t, 2048 * h : 2048 * (h + 1)],
                        scalar1=SS[:, t, 0:1], scalar2=SS[:, t, 1:2],
                        op0=OP.mult, op1=OP.add,
                    )
                # group-0 r chunks fill PE waits on later tiles' GN chains
                for m in range(T):
                    mm(Pr0[m], W1T[:, t, 128 * m : 128 * (m + 1)], XN[:, t, 0:512],
                       start=(t == 0), stop=(t == T - 1))

            # ---- transpose xn -> xnT (XNT reuses X's slot once X is dead) ----
            # transposes are emitted lazily, interleaved into group 0's score
            # loop (PE executes in order: a standalone transpose phase would
            # serialize ~30us before attention can start)
            XNT = pbig.tile([128, JT, C], mdt, tag="big")
            _tp_done = [0]  # j-batches emitted so far (batches of 4 j)

            def emit_transposes(upto_j):
                while _tp_done[0] * 4 < min(upto_j, JT):
                    j0 = _tp_done[0] * 4
                    for t in range(T):
                        PT = ppA.tile([128, 4, 128], mdt, tag="psA", name=f"PT{t}_{j0}")
                        for dj in range(4):
                            nc.tensor.transpose(
                                PT[:, dj, :], XN[:, t, 128 * (j0 + dj) : 128 * (j0 + dj + 1)], IDENT
                            )
                        nc.vector.tensor_copy(XNT[:, j0 : j0 + 4, 128 * t : 128 * (t + 1)], PT)
                    _tp_done[0] += 1

            # ---- attention, per query group of 512 ----
            def emit_r(g, pr_pre=None):
                isl = slice(512 * g, 512 * (g + 1))
                r_sb = prr.tile([128, T, 512], mdt, tag="r", name=f"r{_rep}_{g}")
                for m in range(T):
                    if pr_pre is not None:
                        Pr = pr_pre[m]
                    else:
                        Pr = ppA.tile([128, 512], f32, tag="psA", name=f"Pr{_rep}_{g}_{m}")
                        for t in range(T):
                            mm(Pr, W1T[:, t, 128 * m : 128 * (m + 1)], XN[:, t, isl],
                               start=(t == 0), stop=(t == T - 1))
                    nc.vector.tensor_scalar(
                        out=r_sb[:, m, :], in0=Pr, scalar1=RB[:, m : m + 1],
                        scalar2=None, op0=OP.add,
                    )
                # prefetch residual x for this group
                xrs = []
                for mo in range(T):
                    xr = pio.tile([128, 512], f32, tag="xr", name=f"xr{_rep}_{g}_{mo}", bufs=8)
                    nc.sync.dma_start(out=xr, in_=xb_d[128 * mo : 128 * (mo + 1), isl])
                    xrs.append(xr)
                return r_sb, xrs

            def emit_scores(g, j, r_sb):
                Ps = ppA.tile([128, 512], f32, tag="psA", name=f"Ps{_rep}_{g}_{j}")
                for t in range(T):
                    mm(Ps, XN[:, t, 128 * j : 128 * (j + 1)], r_sb[:, t, :],
                       start=(t == 0), stop=(t == T - 1))
                e = pee.tile([128, 512], mdt, tag="e", name=f"e{_rep}_{g}_{j}")
                nc.scalar.activation(e, Ps, AF.Exp, scale=SCALE)
                return e

            def emit_u(g, j, e, Pu, acc_e):
                for m in range(T):
                    mm(Pu[m], XNT[:, j, 128 * m : 128 * (m + 1)], e,
                       start=(j == 0), stop=(j == JT - 1))
                if denom_mm:
                    mm(acc_e, ONESM, e, start=(j == 0), stop=(j == JT - 1))
                    return
                # accumulate exp on DVE; single denominator matmul per group
                if j == 0:
                    nc.vector.tensor_copy(acc_e, e)
                else:
                    nc.vector.tensor_add(acc_e, acc_e, e)

            def emit_norm(g, Pu, acc_e):
                if denom_mm:
                    Pd = acc_e
                else:
                    # single denominator matmul over the DVE-accumulated exps
                    Pd = ppP.tile([1, 512], f32, tag="psP", name=f"Pd{_rep}_{g}")
                    mm(Pd, ONES, acc_e, start=True, stop=True)
                rec = pio.tile([1, 512], f32, tag="rec", name=f"rec{_rep}_{g}")
                nc.vector.reciprocal(rec, Pd)
                Pb = ppP.tile([128, 512], f32, tag="psP", name=f"Pb{_rep}_{g}")
                mm(Pb, ONESR, rec, start=True, stop=True)
                rbc = pio.tile([128, 512], f32, tag="rbc", name=f"rbc{_rep}_{g}")
                nc.vector.tensor_copy(rbc, Pb)
                u_sb = puu.tile([128, T, 512], mdt, tag="u", name=f"u{_rep}_{g}")
                for m in range(T):
                    nc.vector.tensor_mul(u_sb[:, m, :], Pu[m], rbc)
                return u_sb

            def emit_proj(g, u_sb, xrs):
                isl = slice(512 * g, 512 * (g + 1))
                for mo in range(T):
                    Pp = ppP.tile([128, 512], f32, tag="psP", name=f"Pp{_rep}_{g}_{mo}")
                    for t in range(T):
                        mm(Pp, W2T[:, t, 128 * mo : 128 * (mo + 1)], u_sb[:, t, :],
                           start=(t == 0), stop=(t == T - 1))
                    o = pio.tile([128, 512], f32, tag="o", name=f"o{g}_{mo}", bufs=4)
                    nc.vector.scalar_tensor_tensor(
                        out=o, in0=Pp, scalar=B2[:, mo : mo + 1], in1=xrs[mo],
                        op0=OP.add, op1=OP.add,
                    )
                    nc.sync.dma_start(out=out_d[128 * mo : 128 * (mo + 1), isl], in_=o)

            _rep = -1
            r_sb, xrs = emit_r(0, pr_pre=Pr0)
            for _rep in range(repeat):
              for g in range(IG):
                  Pu = [ppU.tile([128, 512], f32, tag="Pu", name=f"Pu{_rep}_{g}_{m}") for m in range(T)]
                  if denom_mm:
                      acc_e = ppP.tile([1, 512], f32, tag="psP", name=f"Pdm{_rep}_{g}")
                  else:
                      acc_e = pio.tile([128, 512], f32, tag="acc_e", name=f"acc{_rep}_{g}")
                  e_prev = None
                  emit_transposes(8)  # head start for u(j=0..7)
                  for j in range(JT):
                      e = emit_scores(g, j, r_sb)
                      emit_transposes(j + 12)  # stay ~3 batches ahead of u
                      if e_prev is not None:
                          emit_u(g, j - 1, e_prev, Pu, acc_e)
                      e_prev = e
                  emit_u(g, JT - 1, e_prev, Pu, acc_e)
                  # next group's r + residual prefetch fills PE while norm chain runs
                  nxt = g + 1 if g + 1 < IG else (0 if _rep + 1 < repeat else None)
                  if nxt is not None:
                      nr_sb, nxrs = emit_r(nxt)
                  u_sb = emit_norm(g, Pu, acc_e)
                  emit_proj(g, u_sb, xrs)
                  if nxt is not None:
                      r_sb, xrs = nr_sb, nxrs

    nc.compile()
    return nc


def _build_program_fp8(repeat: int = 1):
    """fp8e4m3 DoubleRow variant: all heavy matmuls contract 256 rows/instr at
    0.5 cyc/row. PSUM (8 banks): Ps j-pairs [128,2,512]x2 (4) + Pu x2 (2) +
    Pd denom (1) + rot scratch (1). The attn-V accumulation runs in two
    passes (m=0,1 in-loop; m=2,3 at group end) so it fits 2 banks; the
    denominator is a DoubleRow ones-matmul on the PE; Pool engine takes the
    transpose copies / output assembly off the DVE."""
    _patch_ldw_opt()
    import concourse.bass as bass
    import concourse.tile as tile
    from concourse import bacc, mybir

    f32 = mybir.dt.float32
    fp8 = mybir.dt.float8e4
    AF = mybir.ActivationFunctionType
    OP = mybir.AluOpType
    DR = mybir.MatmulPerfMode.DoubleRow
    JP = JT // 2  # 16 j-pairs

    nc = bacc.Bacc("TRN2")

    xb_d = nc.declare_dram_parameter("xb", [C, N], f32, isOutput=False)
    w1t_d = nc.declare_dram_parameter("w1t", [C, C], fp8, isOutput=False)
    w2t_d = nc.declare_dram_parameter("w2t", [C, C], fp8, isOutput=False)
    cv_d = nc.declare_dram_parameter("cvec", [128, 4, T], f32, isOutput=False)
    gi_d = nc.declare_dram_parameter("gi", [128, 8], f32, isOutput=False)
    git_d = nc.declare_dram_parameter("git", [8, 128], f32, isOutput=False)
    id_d = nc.declare_dram_parameter("ident", [128, 128], fp8, isOutput=False)
    on2_d = nc.declare_dram_parameter("ones2", [128, 2, 128], fp8, isOutput=False)
    out_d = nc.declare_dram_parameter("out", [C, NQ], f32, isOutput=True)

    def mm(out, lhsT, rhs, start, stop):
        nc.tensor.matmul(out, lhsT, rhs, start=start, stop=stop, perf_mode=DR)

    with tile.TileContext(nc) as tc:
        with (
            tc.tile_pool(name="big", bufs=1) as pbig,
            tc.tile_pool(name="const", bufs=1) as pc,
            tc.tile_pool(name="stat", bufs=2) as pst,
            tc.tile_pool(name="rpool", bufs=2) as prr,
            tc.tile_pool(name="upool", bufs=2) as puu,
            tc.tile_pool(name="epool", bufs=17) as pee,
            tc.tile_pool(name="iopool", bufs=2) as pio,
            tc.tile_pool(name="psS", bufs=2, space="PSUM") as ppS,
            tc.tile_pool(name="psU", bufs=2, space="PSUM") as ppU,
            tc.tile_pool(name="psD", bufs=1, space="PSUM") as ppD,
            tc.tile_pool(name="psR", bufs=1, space="PSUM") as ppR,
        ):
            # ---- constant / weight loads ----
            W1T = pc.tile([128, T, C], fp8)
            nc.scalar.dma_start(out=W1T, in_=w1t_d[:].rearrange("(t p) f -> p t f", p=128))
            W2T = pc.tile([128, T, C], fp8)
            nc.scalar.dma_start(out=W2T, in_=w2t_d[:].rearrange("(t p) f -> p t f", p=128))
            GI = pc.tile([128, 8], f32)
            nc.sync.dma_start(out=GI, in_=gi_d[:])
            GIT = pc.tile([8, 128], f32)
            nc.sync.dma_start(out=GIT, in_=git_d[:])
            IDENT = pc.tile([128, 128], fp8)
            nc.sync.dma_start(out=IDENT, in_=id_d[:])
            CV = pc.tile([128, 4, T], f32)
            nc.sync.dma_start(out=CV, in_=cv_d[:])
            RB, B2, GNW, GNB = CV[:, 0, :], CV[:, 1, :], CV[:, 2, :], CV[:, 3, :]
            ONES2 = pc.tile([128, 2, 128], fp8)
            nc.sync.dma_start(out=ONES2, in_=on2_d[:])
            SS = pc.tile([128, T, 2], f32)  # per-channel (scale, shift)
            MB = pc.tile([128, 1], f32)     # exp bias (logit shift)
            nc.vector.memset(MB, -MSHIFT)

            # ---- load x + per-tile GroupNorm ----
            X = pbig.tile([128, T, N], f32, tag="x")       # stays live: residual
            XN = pbig.tile([128, T, N], fp8, tag="xn")
            XNT = pbig.tile([128, JT, C], fp8, tag="xnt")
            # group-0 r accumulates inside the GN loop, borrowing the Ps pair
            # slots (idle until the first scores matmul)
            Pr0p = [ppS.tile([128, 2, 512], f32, tag="Ps", name=f"Pr0p{h}") for h in range(2)]
            Pr0 = [Pr0p[m // 2][:, m % 2, :] for m in range(T)]
            xb_t = xb_d[:].rearrange("(t p) n -> p t n", p=128)
            epsT = pc.tile([8, 1], f32)
            nc.vector.memset(epsT, EPS)
            for t in range(T):
                for h in range(4):
                    eng = nc.scalar if h % 2 == 1 else nc.sync
                    eng.dma_start(
                        out=X[:, t, 1024 * h : 1024 * (h + 1)],
                        in_=xb_t[:, t, 1024 * h : 1024 * (h + 1)],
                    )
                stats_t = pst.tile([128, 8, 6], f32, tag="stats")
                for s in range(8):
                    nc.vector.bn_stats(out=stats_t[:, s, :], in_=X[:, t, 512 * s : 512 * (s + 1)])
                mv_t = pst.tile([128, 2], f32, tag="mv")
                nc.vector.bn_aggr(out=mv_t, in_=stats_t)
                perch_t = pst.tile([128, 2], f32, tag="perch")  # (mean, E[x^2])
                nc.vector.tensor_copy(perch_t[:, 0:1], mv_t[:, 0:1])
                nc.vector.tensor_mul(perch_t[:, 1:2], mv_t[:, 0:1], mv_t[:, 0:1])
                nc.vector.tensor_add(perch_t[:, 1:2], perch_t[:, 1:2], mv_t[:, 1:2])
                GSp = ppR.tile([8, 2], f32, tag="rot", name=f"GSp{t}")
                nc.tensor.matmul(GSp, GI, perch_t, start=True, stop=True)
                GB = pst.tile([8, 2], f32, tag="GB")  # (mean_g, rstd_g)
                tmpg = pst.tile([8, 1], f32, tag="tmpg")
                nc.vector.tensor_copy(GB, GSp)
                nc.vector.tensor_mul(tmpg, GB[:, 0:1], GB[:, 0:1])
                nc.vector.tensor_sub(GB[:, 1:2], GB[:, 1:2], tmpg)  # var_g
                nc.scalar.activation(GB[:, 1:2], GB[:, 1:2], AF.Sqrt, bias=epsT)
                nc.vector.reciprocal(GB[:, 1:2], GB[:, 1:2])
                PB = ppR.tile([128, 2], f32, tag="rot", name=f"PB{t}")
                nc.tensor.matmul(PB, GIT, GB, start=True, stop=True)
                tmpc = pst.tile([128, 1], f32, tag="tmpc")
                nc.vector.tensor_mul(SS[:, t, 0:1], PB[:, 1:2], GNW[:, t : t + 1])
                nc.vector.tensor_mul(tmpc, PB[:, 0:1], SS[:, t, 0:1])
                nc.vector.tensor_sub(SS[:, t, 1:2], GNB[:, t : t + 1], tmpc)
                # xn = x*scale + shift; first half (own queries) on DVE
                # unblocks r, second half on Pool
                for h, eng in ((0, nc.vector), (1, nc.gpsimd)):
                    eng.tensor_scalar(
                        out=XN[:, t, 2048 * h : 2048 * (h + 1)],
                        in0=X[:, t, 2048 * h : 2048 * (h + 1)],
                        scalar1=SS[:, t, 0:1], scalar2=SS[:, t, 1:2],
                        op0=OP.mult, op1=OP.add,
                    )
                # group-0 r: DoubleRow over t-pairs (0,1) and (2,3)
                if t % 2 == 1:
                    tp = t // 2
                    for m in range(T):
                        mm(Pr0[m], W1T[:, t - 1 : t + 1, 128 * m : 128 * (m + 1)],
                           XN[:, t - 1 : t + 1, 0:512],
                           start=(tp == 0), stop=(tp == 1))

            # ---- transposes xn -> xnT, emitted lazily (PE executes in order).
            # XNT channel-chunk m comes from channel-tile t=m; the in-loop u
            # pass only reads m=0,1 so t=0,1 batches are emitted eagerly and
            # t=2,3 lag (needed only by the group-end pass).
            _tp01 = [0]  # j-batches of 4 emitted for t=0,1
            _tp23 = [0]

            def _emit_tp_batch(t, j0, nm):
                # fp8 transpose writes with element step 2 (hw requirement);
                # the copy reads back strided
                PT = ppR.tile([128, 4, 128, 2], fp8, tag="rot", name=f"PT{nm}{t}_{j0}")
                for dj in range(4):
                    nc.tensor.transpose(
                        PT[:, dj, :, 0], XN[:, t, 128 * (j0 + dj) : 128 * (j0 + dj + 1)], IDENT
                    )
                nc.vector.tensor_copy(XNT[:, j0 : j0 + 4, 128 * t : 128 * (t + 1)], PT[:, :, :, 0])

            def emit_transposes01(upto_j):
                while _tp01[0] * 4 < min(upto_j, JT):
                    j0 = _tp01[0] * 4
                    for t in range(2):
                        _emit_tp_batch(t, j0, "a")
                    _tp01[0] += 1

            def emit_transposes23(upto_j):
                while _tp23[0] * 4 < min(upto_j, JT):
                    j0 = _tp23[0] * 4
                    for t in range(2, 4):
                        _emit_tp_batch(t, j0, "b")
                    _tp23[0] += 1

            # ---- attention helpers ----
            def emit_r_chunk(g, m, pr_pre=None):
                """r chunk m for query group g -> r_sb[g%2][:, m, :]."""
                isl = slice(512 * g, 512 * (g + 1))
                if pr_pre is not None:
                    Pr = pr_pre
                else:
                    Pr = ppR.tile([128, 512], f32, tag="rot", name=f"Pr{_rep}_{g}_{m}")
                    for tp in range(2):
                        mm(Pr, W1T[:, 2 * tp : 2 * tp + 2, 128 * m : 128 * (m + 1)],
                           XN[:, 2 * tp : 2 * tp + 2, isl],
                           start=(tp == 0), stop=(tp == 1))
                nc.vector.tensor_scalar(
                    out=_rsb_next[:, m, :], in0=Pr, scalar1=RB[:, m : m + 1],
                    scalar2=None, op0=OP.add,
                )

            def emit_scores(g, jp, r_sb):
                Ps = ppS.tile([128, 2, 512], f32, tag="Ps", name=f"Ps{_rep}_{g}_{jp}")
                for dj in range(2):
                    j = 2 * jp + dj
                    for tp in range(2):
                        mm(Ps[:, dj, :],
                           XN[:, 2 * tp : 2 * tp + 2, 128 * j : 128 * (j + 1)],
                           r_sb[:, 2 * tp : 2 * tp + 2, :],
                           start=(tp == 0), stop=(tp == 1))
                e = pee.tile([128, 2, 512], fp8, tag="e", name=f"e{_rep}_{g}_{jp}")
                nc.scalar.activation(e, Ps, AF.Exp, scale=SCALE, bias=MB)
                return e

            def emit_uA(jp, e, PuA, Pd):
                """in-loop pass: m=0,1 + denominator."""
                for m in range(2):
                    mm(PuA[m], XNT[:, 2 * jp : 2 * jp + 2, 128 * m : 128 * (m + 1)], e,
                       start=(jp == 0), stop=(jp == JP - 1))
                mm(Pd, ONES2, e, start=(jp == 0), stop=(jp == JP - 1))

            def emit_uB(jp, e, PuB):
                for m in range(2, 4):
                    mm(PuB[m - 2], XNT[:, 2 * jp : 2 * jp + 2, 128 * m : 128 * (m + 1)], e,
                       start=(jp == 0), stop=(jp == JP - 1))

            _rep = -1
            # group-0 r from the GN-prologue accumulators
            _rsb_next = prr.tile([128, T, 512], fp8, tag="r", name="r_g0")
            for m in range(T):
                emit_r_chunk(0, m, pr_pre=Pr0[m])
            r_sb = _rsb_next

            for _rep in range(repeat):
              for g in range(IG):
                PuA = [ppU.tile([128, 512], f32, tag="Pu", name=f"PuA{_rep}_{g}_{m}") for m in range(2)]
                Pd = ppD.tile([128, 512], f32, tag="Pd", name=f"Pd{_rep}_{g}")
                es = []
                e_prev = None
                nxt = g + 1 if g + 1 < IG else (0 if _rep + 1 < repeat else None)
                if g == 0 and _rep == 0:
                    emit_transposes01(8)
                for jp in range(JP):
                    e = emit_scores(g, jp, r_sb)
                    es.append(e)
                    if g == 0 and _rep == 0:
                        emit_transposes01(2 * jp + 12)
                        emit_transposes23(2 * jp + 4)
                    if e_prev is not None:
                        emit_uA(jp - 1, e_prev, PuA, Pd)
                    e_prev = e
                    # next group's r, spread across the loop middle
                    if nxt is not None and jp in (4, 6, 8, 10):
                        if jp == 4:
                            _rsb_next = prr.tile([128, T, 512], fp8, tag="r",
                                                 name=f"r{_rep}_{g}_nxt")
                        emit_r_chunk(nxt, (jp - 4) // 2)
                if g == 0 and _rep == 0:
                    emit_transposes23(JT)
                # group-end pass m=2,3: pairs 0..14 fill the PE while the last
                # exp drains on Act, then uA/uB for the final pair
                PuB = [ppU.tile([128, 512], f32, tag="Pu", name=f"PuB{_rep}_{g}_{m}") for m in range(2)]
                for jp in range(JP - 1):
                    emit_uB(jp, es[jp], PuB)
                emit_uA(JP - 1, e_prev, PuA, Pd)
                emit_uB(JP - 1, es[JP - 1], PuB)

                # ---- normalize + project + store ----
                isl = slice(512 * g, 512 * (g + 1))
                rbc = pio.tile([128, 512], f32, tag="rbc", name=f"rbc{_rep}_{g}")
                nc.vector.reciprocal(rbc, Pd)
                u_sb = puu.tile([128, T, 512], fp8, tag="u", name=f"u{_rep}_{g}")
                for m in range(2):
                    nc.vector.tensor_mul(u_sb[:, m, :], PuA[m], rbc)
                for m in range(2, 4):
                    nc.vector.tensor_mul(u_sb[:, m, :], PuB[m - 2], rbc)
                for mo in range(T):
                    Pp = ppR.tile([128, 512], f32, tag="rot", name=f"Pp{_rep}_{g}_{mo}")
                    for tp in range(2):
                        mm(Pp, W2T[:, 2 * tp : 2 * tp + 2, 128 * mo : 128 * (mo + 1)],
                           u_sb[:, 2 * tp : 2 * tp + 2, :],
                           start=(tp == 0), stop=(tp == 1))
                    o = pio.tile([128, 512], f32, tag="o", name=f"o{_rep}_{g}_{mo}", bufs=4)
                    nc.vector.scalar_tensor_tensor(
                        out=o, in0=Pp, scalar=B2[:, mo : mo + 1], in1=X[:, mo, isl],
                        op0=OP.add, op1=OP.add,
                    )
                    nc.sync.dma_start(out=out_d[128 * mo : 128 * (mo + 1), isl], in_=o)
                if nxt is not None:
                    r_sb = _rsb_next

    nc.compile()
    return nc


def _host_inputs(x, gn_w, gn_b, wq, bq, wk, bk, wv, bv, wp, bp, mm_dtype_name=None):
    """Host-side weight fusion (fp64) + per-core input maps."""
    f32 = np.float32
    if mm_dtype_name is None:
        mm_dtype_name = MM_DTYPE
    if mm_dtype_name == "bf16":
        import ml_dtypes
        mmnp = ml_dtypes.bfloat16
    elif mm_dtype_name == "fp8":
        import ml_dtypes
        mmnp = ml_dtypes.float8_e4m3
    else:
        mmnp = np.float32
    wq64, wk64, wv64, wp64 = (np.asarray(w, np.float64) for w in (wq, wk, wv, wp))
    w1t = (wq64.T @ wk64).astype(f32)                      # [c'', c']
    w2t = (np.asarray(wp, np.float64) @ wv64).T.astype(f32)  # [c', c_out]
    rb = (wk64.T @ np.asarray(bq, np.float64)).astype(f32)   # [c']
    b2 = (wp64 @ np.asarray(bv, np.float64) + np.asarray(bp, np.float64)).astype(f32)

    def tile_vec(v):
        return np.ascontiguousarray(np.asarray(v, f32).reshape(T, 128).T)

    gs = C // NUM_GROUPS  # 16 channels per group; 8 local groups per 128-chan tile
    gi = np.zeros((128, 8), f32)
    git = np.zeros((8, 128), f32)
    for p in range(128):
        gi[p, p // gs] = 1.0 / gs  # group stat = mean of the 16 per-channel stats
        git[p // gs, p] = 1.0
    ident = np.eye(128, dtype=f32)

    cvec = np.ascontiguousarray(
        np.stack([tile_vec(rb), tile_vec(b2), tile_vec(gn_w), tile_vec(gn_b)], axis=1)
    )
    common = {
        "w1t": w1t.astype(mmnp),
        "w2t": np.ascontiguousarray(w2t).astype(mmnp),
        "cvec": cvec,
        "gi": gi,
        "git": git,
        "ident": ident.astype(mmnp),
        "onesr": np.ones((1, 128), np.float32),
    }
    if mm_dtype_name == "fp8":
        common["ones2"] = np.ones((128, 2, 128), mmnp)
    else:
        common["onesc"] = np.ones((128, 1), np.float32)

    x2 = np.asarray(x, f32).reshape(B, C, N)
    in_maps = []
    for core in range(8):
        b, s = divmod(core, 2)
        xb = x2[b]
        if s == 1:
            xb = np.concatenate([xb[:, NQ:], xb[:, :NQ]], axis=1)
        m = dict(common)
        m["xb"] = np.ascontiguousarray(xb)
        in_maps.append(m)
    return in_maps


def kernel(**inputs):
    global LAST_RESULTS
    from concourse.bass_utils import run_bass_kernel_spmd

    key = MM_DTYPE
    if key not in _PROGRAM_CACHE:
        if key == "fp8":
            _PROGRAM_CACHE[key] = _build_program_fp8()
        else:
            _PROGRAM_CACHE[key] = _build_program(key)
    nc = _PROGRAM_CACHE[key]

    in_maps = _host_inputs(**{k: np.asarray(v) for k, v in inputs.items()})
    trace = bool(int(os.environ.get("BASS_KERNEL_TRACE", "0")))
    res = run_bass_kernel_spmd(
        nc, in_maps, list(range(8)), trace=trace,
        trace_cores=list(range(8)) if trace else None,
    )
    LAST_RESULTS = res

    out = np.empty((B, C, N), np.float32)
    for core in range(8):
        b, s = divmod(core, 2)
        out[b, :, NQ * s : NQ * (s + 1)] = res.results[core]["out"]
    return out.reshape(B, C, H, W)

